# revision 3
# baseline (speedup 1.0000x reference)
"""AWLoss1D batched-Toeplitz-solve loss on 8 Trainium2 NeuronCores.

Math (per batch row b of 512):
  D_b = (511x256) Toeplitz of target_b;  A_b = D^T D + eps*I;
  v_b = A_b^{-1} (D^T pad(recon_b));  loss = sum_b 0.5*||T.v||/||v||.

Device algorithm (64 systems per core, pure data parallel):
  * A_b embeds in the 512-circulant with eigenvalues lam_b =
    |FFT_512(target_b zero-padded)|^2; lam symmetric => diagonalized by the
    real 512-point Hartley transform H5 shared by all batches, so batched
    matvecs are plain PE matmuls with batch on the free dim.
  * Two-step CHEBYSHEV-style iteration with data-independent scalars
    (alpha0, alpha1, beta1) tuned numerically on the reference model —
    no dot products, no reciprocals, no per-batch scalar broadcasts.
    All spectral state: xh/ph 512-spectra, rh 256-Hartley spectra;
    KM = (1/512) H2 H5^T and K2 = (1/256) H5 H2^T map between domains.
  * Preconditioner: blended Chan/Strang circulant, spectrum mu from
    lam via a host-precomputed 256x512 map W.
  * Matmul dtypes: moving operands bf16 (1 PE cycle/row vs 4 for f32);
    setup-only weights (FFT/B/W) fp8 with power-of-2 prescales folded
    into host constants and immediates; KM/K2/IH5 weights bf16.
    lam64 = lam/64 is the only stored spectrum (the 64 folds into the
    residual-update immediates).
  * RHS spectrum directly: bh = B64c@(Zre/64) + B64s@(Zimn/64) where
    Z = conj(FFT(target)).FFT(recon), pad-127 shift folded into B on host.
  * Finale: v^T = IH5^T (x0 + (a0+a1*b1) p0 + a1 k2) as one fused update;
    per-batch ||Tv||^2/||v||^2 via ACT Square + ones-matmul partition
    reductions; per-batch 0.5*sqrt on ACT; host sums the 8x64 partials.
"""
import functools

import numpy as np

B, HH, N, NCORES = 512, 256, 512, 8
BPC = B // NCORES  # 64 batches per core
EPS = 1e-4
FLOOR = 0.1

# Chebyshev-style scalars, tuned on the f64 reference with the exact
# device rounding model (robust to +-2% perturbation).
AL0 = 0.18383249176451916
AL1 = 0.3939614782927838
BE1 = 0.2285507684190372
XS = AL0 + AL1 * BE1


def _bf16np():
    import ml_dtypes
    return ml_dtypes.bfloat16


def _fp8np():
    import ml_dtypes
    return ml_dtypes.float8_e4m3


@functools.lru_cache(maxsize=1)
def _host_consts():
    """Constant matrices in f64, quantized and pre-swizzled to the
    [128, chunks*cols] per-partition-contiguous DMA layout."""
    bf16 = _bf16np()
    fp8 = _fp8np()

    n5 = np.arange(N)
    n2 = np.arange(HH)
    ang5 = 2.0 * np.pi * np.outer(n5, n5) / N
    cas5 = np.cos(ang5) + np.sin(ang5)
    ang2 = 2.0 * np.pi * np.outer(n2, n2) / HH
    cas2 = np.cos(ang2) + np.sin(ang2)
    H5 = cas5[:, :HH]                                   # [512 f, 256 n]
    H2 = cas2                                           # [256 g, 256 n]

    KMT = ((H2 @ H5.T) / N).T.copy()                    # lhsT [512 f, 256 g]
    K2T = ((H5 @ H2.T) / HH).T.copy()                   # lhsT [256 g, 512 f]
    IH5 = (cas5 / N)[:, :HH].copy()                     # lhsT [512 f, 256 n]
    FCT = (8.0 * np.cos(ang5))[:, :HH].T.copy()         # lhsT [256 n, 512 f]
    FST = (-8.0 * np.sin(ang5))[:, :HH].T.copy()
    angb = 2.0 * np.pi * np.outer(n5, n2 - 127.0) / N   # [f, j]
    BCm = np.cos(angb) / N
    BSm = np.sin(angb) / N
    BCHT = (64.0 * (H2 @ BCm.T)).T.copy()               # lhsT [512 f, 256 g]
    BSHT = (64.0 * (H2 @ BSm.T)).T.copy()
    # preconditioner spectrum map: 0.35/0.65 Chan/Strang circulant blend
    RHO = np.cos(2.0 * np.pi * np.outer(n2, n5) / N) / N
    CW_chan = np.zeros((HH, HH))
    CW_chan[n2, n2] += (HH - n2) / HH
    CW_chan[n2, (HH - n2) % HH] += n2 / HH
    CW_str = np.zeros((HH, HH))
    CW_str[n2, n2] += 1.0
    CW_str[n2[1:], (HH - n2[1:]) % HH] += 1.0
    CW = 0.35 * CW_chan + 0.65 * CW_str
    DCT = np.cos(2.0 * np.pi * np.outer(n2, n2) / HH)
    W = DCT @ CW @ RHO                                  # [256 g, 512 f]
    WT = (64.0 * W).T.copy()                            # lhsT [512 f, 256 g]
    cv = (EPS * (1.0 - W.sum(axis=1)))[:, None].copy()  # [256 g, 1]

    x = np.linspace(-10.0, 10.0, HH)
    dx = (x[-1] - x[0]) / (HH - 1)
    dispx = (HH % 2 - 1) / 2.0
    g = -np.exp(-((x - dx * dispx) ** 2) / 2.0)
    g = g + np.max(np.abs(g))
    Tw = ((g / np.max(np.abs(g))) ** 2)[:, None].copy()  # T^2, [256 n, 1]

    def swz(a, dt):
        """[C*128, X] lhsT -> [128, C*X] with partition rows contiguous."""
        a = np.asarray(a, dtype=np.float32)
        c = a.shape[0] // 128
        return np.ascontiguousarray(
            a.reshape(c, 128, a.shape[1]).transpose(1, 0, 2).reshape(
                128, c * a.shape[1])).astype(dt)

    return {
        "fc8": swz(FCT, fp8), "fs8": swz(FST, fp8),
        "w64": swz(WT, fp8), "b64c": swz(BCHT, fp8), "b64s": swz(BSHT, fp8),
        "k2t": swz(K2T, bf16), "kmt": swz(KMT, bf16), "ih5": swz(IH5, bf16),
        "tn": swz(Tw, np.float32),
    }


@functools.lru_cache(maxsize=1)
def _program():
    import concourse.bacc as bacc
    import concourse.mybir as mybir
    import concourse.tile as tile

    F32 = mybir.dt.float32
    BF16 = mybir.dt.bfloat16
    FP8 = mybir.dt.float8e4
    AL = mybir.AluOpType
    ACTF = mybir.ActivationFunctionType

    nc = bacc.Bacc(target_bir_lowering=False)

    d_trh = nc.dram_tensor("trh", [128, 2 * 128], BF16, kind="ExternalInput")
    dm = {}
    for name, cols, dt in [
        ("fc8", 2 * N, FP8), ("fs8", 2 * N, FP8), ("w64", 4 * HH, FP8),
        ("b64c", 4 * HH, FP8), ("b64s", 4 * HH, FP8), ("k2t", 2 * N, BF16),
        ("kmt", 4 * HH, BF16), ("ih5", 4 * HH, BF16),
        ("tn", 2 * 1, F32),
    ]:
        dm[name] = nc.dram_tensor(name, [128, cols], dt, kind="ExternalInput")
    d_out = nc.dram_tensor("out", [1, BPC], F32, kind="ExternalOutput")

    with tile.TileContext(nc) as tc:
        with (
            tc.tile_pool(name="consts", bufs=1) as consts,
            tc.tile_pool(name="state", bufs=1) as state,
            tc.tile_pool(name="psum", bufs=1, space="PSUM") as psum,
        ):
            def loadc(name, chunks, eng):
                cols = dm[name].shape[1] // chunks
                t = consts.tile([128, chunks, cols], dm[name].dtype, tag=name)
                eng.dma_start(
                    out=t,
                    in_=dm[name].ap().rearrange("p (c x) -> p c x", c=chunks))
                return t

            # ---- DMA issue order chosen so the globally-serialized DMA
            # transfers run in deadline order:
            # trh, fc8, fs8, b64c, cv, b64s, w64, k2t, kmt, tn, ih5 ----
            tr = state.tile([128, 2, 128], BF16, tag="tr")
            nc.gpsimd.dma_start(
                out=tr, in_=d_trh.ap().rearrange("p (c x) -> p c x", c=2))
            fc8 = loadc("fc8", 2, nc.sync)       # SP#1
            fs8 = loadc("fs8", 2, nc.scalar)     # ACT#1
            b64c = loadc("b64c", 4, nc.sync)     # SP#2
            b64s = loadc("b64s", 4, nc.scalar)   # ACT#2
            w64 = loadc("w64", 4, nc.sync)       # SP#3
            k2t = loadc("k2t", 2, nc.scalar)     # ACT#3
            kmt = loadc("kmt", 4, nc.sync)       # SP#4
            ih5 = loadc("ih5", 4, nc.sync)       # SP#5
            tnt = loadc("tn", 2, nc.gpsimd)      # Pool#2

            ones = consts.tile([128, 1], BF16, tag="ones")
            nc.gpsimd.memset(ones, 1.0)
            # warm the ACT Square+Sqrt tables off the critical path
            # (~1.3us per set load). The warm input must be f32 — table
            # sets are input-dtype-specific and the real uses are f32.
            onesf = consts.tile([1, 1], F32, tag="onesf")
            nc.gpsimd.memset(onesf, 1.0)
            sqwarm = consts.tile([1, 2], F32, tag="sqwarm")
            nc.scalar.activation(
                out=sqwarm[:, 0:1], in_=onesf, func=ACTF.Square, scale=1.0)
            nc.scalar.activation(
                out=sqwarm[:, 1:2], in_=onesf, func=ACTF.Sqrt, scale=1.0)

            # ---- FFT of [t | r]: re/im = FC/FS @ tr  (free dim 128) ----
            re_ps = psum.tile([128, 4, 128], F32, tag="re")
            im_ps = psum.tile([128, 4, 128], F32, tag="im")
            for ps, w in ((re_ps, fc8), (im_ps, fs8)):
                for ot in range(4):
                    for kc in range(2):
                        nc.tensor.matmul(
                            ps[:, ot, :], w[:, kc, ot * 128:(ot + 1) * 128],
                            tr[:, kc, :], start=(kc == 0), stop=(kc == 1))
            ure = re_ps[:, :, 0:BPC]
            rre = re_ps[:, :, BPC:2 * BPC]
            uim = im_ps[:, :, 0:BPC]
            rim = im_ps[:, :, BPC:2 * BPC]

            # ---- Engine legality on real HW: GPSIMD (Pool) cannot
            # access PSUM at all; DVE/ACT ops may read at most one PSUM
            # operand. U halves are copied to SBUF (DVE: ure, ACT: uim
            # and rim); products read one PSUM side each; squares and
            # lam64 run on Pool from SBUF. Cross-engine readers of one
            # PSUM tile serialize in declaration order, so readers are
            # declared most-critical-first. ----
            ureb = state.tile([128, 4, BPC], F32, tag="ureb")
            uimb = state.tile([128, 4, BPC], F32, tag="uimb")
            rimb = state.tile([128, 4, BPC], F32, tag="rimb")
            nc.vector.tensor_copy(ureb, ure)
            nc.scalar.copy(uimb, uim)
            nc.scalar.copy(rimb, rim)
            S64 = 1.0 / 64.0
            sqre = state.tile([128, 4, BPC], BF16, tag="sqre")
            sqim = state.tile([128, 4, BPC], BF16, tag="sqim")
            nc.gpsimd.scalar_tensor_tensor(
                out=sqre, in0=ureb, scalar=S64, in1=ureb, op0=AL.mult,
                op1=AL.mult)
            nc.gpsimd.scalar_tensor_tensor(
                out=sqim, in0=uimb, scalar=S64, in1=uimb, op0=AL.mult,
                op1=AL.mult)
            lam64 = state.tile([128, 4, BPC], BF16, tag="lam64")
            nc.gpsimd.scalar_tensor_tensor(
                out=lam64, in0=sqre, scalar=EPS / 64.0, in1=sqim,
                op0=AL.add, op1=AL.add)
            t1 = state.tile([128, 4, BPC], BF16, tag="t1")
            t2 = state.tile([128, 4, BPC], BF16, tag="t2")
            t3 = state.tile([128, 4, BPC], BF16, tag="t3")
            t4n = state.tile([128, 4, BPC], BF16, tag="t4n")
            nc.vector.scalar_tensor_tensor(
                out=t1, in0=ureb, scalar=S64, in1=rre, op0=AL.mult,
                op1=AL.mult)
            nc.vector.scalar_tensor_tensor(
                out=t3, in0=uimb, scalar=S64, in1=rre, op0=AL.mult,
                op1=AL.mult)
            nc.gpsimd.scalar_tensor_tensor(
                out=t2, in0=uimb, scalar=S64, in1=rimb, op0=AL.mult,
                op1=AL.mult)
            nc.gpsimd.scalar_tensor_tensor(
                out=t4n, in0=ureb, scalar=-S64, in1=rimb, op0=AL.mult,
                op1=AL.mult)

            # ---- lc = W64@lam64 first (lam64 ready early), then
            # bh = B64c@(t1+t2) + B64s@(t3+t4n) as one chain per gtile
            # (one open PSUM accumulation group per bank at a time);
            # late products (t2/t4n from Pool) ordered last per chain ----
            bha_ps = psum.tile([128, BPC], F32, tag="pb")
            bhb_ps = psum.tile([128, BPC], F32, tag="pg")
            lc_ps = psum.tile([128, 2, BPC], F32, tag="pa")
            for gt in range(2):
                for si, sq in enumerate((sqre, sqim)):
                    for kc in range(4):
                        nc.tensor.matmul(
                            lc_ps[:, gt, :],
                            w64[:, kc, gt * 128:(gt + 1) * 128],
                            sq[:, kc, :], start=(si == 0 and kc == 0),
                            stop=(si == 1 and kc == 3))
            terms = ((b64c, t1), (b64c, t2), (b64s, t3), (b64s, t4n))
            for ti, (w, t) in enumerate(terms):
                for gt, ps in ((0, bha_ps), (1, bhb_ps)):
                    for fc_ in range(4):
                        nc.tensor.matmul(
                            ps, w[:, fc_, gt * 128:(gt + 1) * 128],
                            t[:, fc_, :],
                            start=(ti == 0 and fc_ == 0),
                            stop=(ti == 3 and fc_ == 3))

            # ---- mu = 1/max(W@lam + cv, FLOOR) ----
            mu01 = state.tile([128, 2, BPC], F32, tag="mu01")
            mu = state.tile([128, 2, BPC], F32, tag="mu")
            nc.vector.tensor_scalar(
                out=mu01, in0=lc_ps, scalar1=EPS, scalar2=FLOOR,
                op0=AL.add, op1=AL.max)
            nc.vector.reciprocal(mu, mu01)

            # ---- x0 = K2(mu.bh); r0 = bh - 64 KM(lam64.x0); p0 = K2(mu.r0)

            def mm_k2(src_b, ptag):
                ps = psum.tile([128, 4, BPC], F32, tag=ptag)
                for ot in range(4):
                    for gc in range(2):
                        nc.tensor.matmul(
                            ps[:, ot, :],
                            k2t[:, gc, ot * 128:(ot + 1) * 128],
                            src_b[:, gc, :], start=(gc == 0), stop=(gc == 1))
                return ps

            def mm_km(src_b, ptag):
                ps = psum.tile([128, 2, BPC], F32, tag=ptag)
                for gt in range(2):
                    for fc_ in range(4):
                        nc.tensor.matmul(
                            ps[:, gt, :],
                            kmt[:, fc_, gt * 128:(gt + 1) * 128],
                            src_b[:, fc_, :], start=(fc_ == 0),
                            stop=(fc_ == 3))
                return ps

            sh0 = state.tile([128, 2, BPC], BF16, tag="sh0")
            nc.vector.tensor_mul(sh0[:, 0, :], mu[:, 0, :], bha_ps)
            nc.vector.tensor_mul(sh0[:, 1, :], mu[:, 1, :], bhb_ps)
            x0_ps = mm_k2(sh0, "pc")

            th0 = state.tile([128, 4, BPC], BF16, tag="th0")
            nc.vector.tensor_mul(th0, lam64, x0_ps)
            bhs = state.tile([128, 2, BPC], F32, tag="bhs")
            nc.scalar.copy(bhs[:, 0, :], bha_ps)
            nc.scalar.copy(bhs[:, 1, :], bhb_ps)
            x0s = state.tile([128, 4, BPC], F32, tag="x0s")
            nc.scalar.copy(x0s, x0_ps)
            g0_ps = mm_km(th0, "pa")
            rh = state.tile([128, 2, BPC], F32, tag="rh")
            nc.vector.scalar_tensor_tensor(
                out=rh, in0=g0_ps, scalar=-64.0, in1=bhs, op0=AL.mult,
                op1=AL.add)
            sh = state.tile([128, 2, BPC], BF16, tag="sh")
            nc.vector.tensor_mul(sh, mu, rh)
            p0_ps = mm_k2(sh, "pd")

            # ---- one full iteration (constant scalars) ----
            th = state.tile([128, 4, BPC], BF16, tag="th")
            nc.vector.tensor_mul(th, lam64, p0_ps)
            p0s = state.tile([128, 4, BPC], F32, tag="p0s")
            nc.scalar.copy(p0s, p0_ps)
            gh_ps = mm_km(th, "pa")
            rh2 = state.tile([128, 2, BPC], F32, tag="rh2")
            nc.vector.scalar_tensor_tensor(
                out=rh2, in0=gh_ps, scalar=-64.0 * AL0, in1=rh, op0=AL.mult,
                op1=AL.add)
            sh2 = state.tile([128, 2, BPC], BF16, tag="sh2")
            nc.vector.tensor_mul(sh2, mu, rh2)
            k2_ps = mm_k2(sh2, "pf")

            # xh2 = x0 + (a0 + a1 b1) p0 on Pool, off the critical path
            xh2 = state.tile([128, 4, BPC], F32, tag="xh2")
            nc.gpsimd.scalar_tensor_tensor(
                out=xh2, in0=p0s, scalar=XS, in1=x0s, op0=AL.mult,
                op1=AL.add)

            u = state.tile([128, 4, BPC], BF16, tag="u")
            nc.vector.scalar_tensor_tensor(
                out=u, in0=k2_ps, scalar=AL1, in1=xh2, op0=AL.mult,
                op1=AL.add)

            # ---- finale: vN = IH5^T u (n-partition layout), ratios ----
            vN_ps = psum.tile([128, 2, BPC], F32, tag="im")
            for nt in range(2):
                for fc_ in range(4):
                    nc.tensor.matmul(
                        vN_ps[:, nt, :],
                        ih5[:, fc_, nt * 128:(nt + 1) * 128], u[:, fc_, :],
                        start=(fc_ == 0), stop=(fc_ == 3))
            sqD = state.tile([128, 2, BPC], BF16, tag="sqD")
            nc.scalar.activation(out=sqD, in_=vN_ps, func=ACTF.Square,
                                 scale=1.0)
            # sqT = (T.v)^2 = T^2 * sqD — stays on DVE, reads SBUF
            sqT = state.tile([128, 2, BPC], BF16, tag="sqT")
            for nt in range(2):
                nc.gpsimd.tensor_scalar(
                    out=sqT[:, nt, :], in0=sqD[:, nt, :],
                    scalar1=tnt[:, nt, :], scalar2=None, op0=AL.mult)
            den2_ps = psum.tile([1, BPC], F32, tag="pc")
            num2_ps = psum.tile([1, BPC], F32, tag="pa")
            for nt in range(2):
                nc.tensor.matmul(den2_ps, ones, sqD[:, nt, :],
                                 start=(nt == 0), stop=(nt == 1))
            for nt in range(2):
                nc.tensor.matmul(num2_ps, ones, sqT[:, nt, :],
                                 start=(nt == 0), stop=(nt == 1))
            den2s = state.tile([1, BPC], F32, tag="den2s")
            nc.vector.tensor_copy(den2s, den2_ps)
            rat = state.tile([1, BPC], F32, tag="rat")
            nc.vector.tensor_tensor(
                out=rat, in0=num2_ps, in1=den2s, op=AL.divide)
            srat = state.tile([1, BPC], F32, tag="srat")
            nc.scalar.activation(out=srat, in_=rat, func=ACTF.Sqrt,
                                 scale=0.25)
            nc.sync.dma_start(out=d_out.ap(), in_=srat)

    nc.finalize()
    return nc


def _pack_inputs(recon, target):
    """Per-core [128, 256] bf16 DMA payloads: inputs prescaled by 1/8
    (exact) to match the x8 FFT weight prescale; partition p row c holds
    [target[:, c*128+p] | recon[:, c*128+p]]."""
    bf16 = _bf16np()
    outs = []
    for c in range(NCORES):
        sl = slice(c * BPC, (c + 1) * BPC)
        tt = (target[sl].astype(np.float32) * 0.125).astype(bf16)
        rr = (recon[sl].astype(np.float32) * 0.125).astype(bf16)
        tr3 = np.empty((128, 2, 2 * BPC), dtype=bf16)
        for kc in range(2):
            tr3[:, kc, 0:BPC] = tt[:, kc * 128:(kc + 1) * 128].T
            tr3[:, kc, BPC:2 * BPC] = rr[:, kc * 128:(kc + 1) * 128].T
        outs.append(np.ascontiguousarray(tr3.reshape(128, 2 * 128)))
    return outs


def kernel(recon: np.ndarray, target: np.ndarray) -> np.ndarray:
    from concourse.bass_utils import run_bass_kernel_spmd

    consts = _host_consts()
    nc = _program()

    trhs = _pack_inputs(recon, target)
    in_maps = []
    for c in range(NCORES):
        m = dict(consts)
        m["trh"] = trhs[c]
        in_maps.append(m)

    res = run_bass_kernel_spmd(nc, in_maps, core_ids=list(range(NCORES)))
    kernel._last_results = res  # for test.py introspection (profiling)
    total = 0.0
    for c in range(NCORES):
        total += float(res.results[c]["out"].astype(np.float64).sum())
    return np.float32(total)


# revision 6
# speedup vs baseline: 1.0539x; 1.0539x over previous
"""AWLoss1D batched-Toeplitz-solve loss on 8 Trainium2 NeuronCores.

Math (per batch row b of 512):
  D_b = (511x256) Toeplitz of target_b;  A_b = D^T D + eps*I;
  v_b = A_b^{-1} (D^T pad(recon_b));  loss = sum_b 0.5*||T.v||/||v||.

Device algorithm (64 systems per core, pure data parallel):
  * A_b embeds in the 512-circulant with eigenvalues lam_b =
    |FFT_512(target_b zero-padded)|^2; lam symmetric => diagonalized by the
    real 512-point Hartley transform H5 shared by all batches, so batched
    matvecs are plain PE matmuls with batch on the free dim.
  * Two-step CHEBYSHEV-style iteration with data-independent scalars
    (alpha0, alpha1, beta1) tuned numerically on the reference model —
    no dot products, no reciprocals, no per-batch scalar broadcasts.
    All spectral state: xh/ph 512-spectra, rh 256-Hartley spectra;
    KM = (1/512) H2 H5^T and K2 = (1/256) H5 H2^T map between domains.
  * Preconditioner: blended Chan/Strang circulant, spectrum mu from
    lam via a host-precomputed 256x512 map W.
  * Matmul dtypes: moving operands bf16 (1 PE cycle/row vs 4 for f32);
    setup-only weights (FFT/B/W) fp8 with power-of-2 prescales folded
    into host constants and immediates; KM/K2/IH5 weights bf16.
    lam64 = lam/64 is the only stored spectrum (the 64 folds into the
    residual-update immediates).
  * RHS spectrum directly: bh = B64c@(Zre/64) + B64s@(Zimn/64) where
    Z = conj(FFT(target)).FFT(recon), pad-127 shift folded into B on host.
  * Finale: v^T = IH5^T (x0 + (a0+a1*b1) p0 + a1 k2) as one fused update;
    per-batch ||Tv||^2/||v||^2 via ACT Square + ones-matmul partition
    reductions; per-batch 0.5*sqrt on ACT; host sums the 8x64 partials.
"""
import functools

import numpy as np

B, HH, N, NCORES = 512, 256, 512, 8
BPC = B // NCORES  # 64 batches per core
EPS = 1e-4
FLOOR = 0.1

# Chebyshev-style scalars, tuned on the f64 reference with the exact
# device rounding model (robust to +-2% perturbation).
AL0 = 0.18383249176451916
AL1 = 0.3939614782927838
BE1 = 0.2285507684190372
XS = AL0 + AL1 * BE1


def _bf16np():
    import ml_dtypes
    return ml_dtypes.bfloat16


def _fp8np():
    import ml_dtypes
    return ml_dtypes.float8_e4m3


@functools.lru_cache(maxsize=1)
def _host_consts():
    """Constant matrices in f64, quantized and pre-swizzled to the
    [128, chunks*cols] per-partition-contiguous DMA layout."""
    bf16 = _bf16np()
    fp8 = _fp8np()

    n5 = np.arange(N)
    n2 = np.arange(HH)
    ang5 = 2.0 * np.pi * np.outer(n5, n5) / N
    cas5 = np.cos(ang5) + np.sin(ang5)
    ang2 = 2.0 * np.pi * np.outer(n2, n2) / HH
    cas2 = np.cos(ang2) + np.sin(ang2)
    H5 = cas5[:, :HH]                                   # [512 f, 256 n]
    H2 = cas2                                           # [256 g, 256 n]

    KMT = ((H2 @ H5.T) / N).T.copy()                    # lhsT [512 f, 256 g]
    K2T = ((H5 @ H2.T) / HH).T.copy()                   # lhsT [256 g, 512 f]
    IH5 = (cas5 / N)[:, :HH].copy()                     # lhsT [512 f, 256 n]
    FCT = (8.0 * np.cos(ang5))[:, :HH].T.copy()         # lhsT [256 n, 512 f]
    FST = (-8.0 * np.sin(ang5))[:, :HH].T.copy()
    angb = 2.0 * np.pi * np.outer(n5, n2 - 127.0) / N   # [f, j]
    BCm = np.cos(angb) / N
    BSm = np.sin(angb) / N
    BCHT = (64.0 * (H2 @ BCm.T)).T.copy()               # lhsT [512 f, 256 g]
    BSHT = (64.0 * (H2 @ BSm.T)).T.copy()
    # preconditioner spectrum map: 0.35/0.65 Chan/Strang circulant blend
    RHO = np.cos(2.0 * np.pi * np.outer(n2, n5) / N) / N
    CW_chan = np.zeros((HH, HH))
    CW_chan[n2, n2] += (HH - n2) / HH
    CW_chan[n2, (HH - n2) % HH] += n2 / HH
    CW_str = np.zeros((HH, HH))
    CW_str[n2, n2] += 1.0
    CW_str[n2[1:], (HH - n2[1:]) % HH] += 1.0
    CW = 0.35 * CW_chan + 0.65 * CW_str
    DCT = np.cos(2.0 * np.pi * np.outer(n2, n2) / HH)
    W = DCT @ CW @ RHO                                  # [256 g, 512 f]
    WT = (64.0 * W).T.copy()                            # lhsT [512 f, 256 g]
    cv = (EPS * (1.0 - W.sum(axis=1)))[:, None].copy()  # [256 g, 1]

    x = np.linspace(-10.0, 10.0, HH)
    dx = (x[-1] - x[0]) / (HH - 1)
    dispx = (HH % 2 - 1) / 2.0
    g = -np.exp(-((x - dx * dispx) ** 2) / 2.0)
    g = g + np.max(np.abs(g))
    Tw = ((g / np.max(np.abs(g))) ** 2)[:, None].copy()  # T^2, [256 n, 1]

    def swz(a, dt):
        """[C*128, X] lhsT -> [128, C*X] with partition rows contiguous."""
        a = np.asarray(a, dtype=np.float32)
        c = a.shape[0] // 128
        return np.ascontiguousarray(
            a.reshape(c, 128, a.shape[1]).transpose(1, 0, 2).reshape(
                128, c * a.shape[1])).astype(dt)

    return {
        "fc8": swz(FCT, fp8), "fs8": swz(FST, fp8),
        "w64": swz(WT, fp8), "b64c": swz(BCHT, fp8), "b64s": swz(BSHT, fp8),
        "k2t": swz(K2T, bf16), "kmt": swz(KMT, bf16), "ih5": swz(IH5, bf16),
        "tn": swz(Tw, np.float32),
    }


@functools.lru_cache(maxsize=1)
def _program():
    import concourse.bacc as bacc
    import concourse.mybir as mybir
    import concourse.tile as tile

    F32 = mybir.dt.float32
    BF16 = mybir.dt.bfloat16
    FP8 = mybir.dt.float8e4
    AL = mybir.AluOpType
    ACTF = mybir.ActivationFunctionType

    nc = bacc.Bacc(target_bir_lowering=False)

    d_trh = nc.dram_tensor("trh", [128, 2 * 128], BF16, kind="ExternalInput")
    dm = {}
    for name, cols, dt in [
        ("fc8", 2 * N, FP8), ("fs8", 2 * N, FP8), ("w64", 4 * HH, FP8),
        ("b64c", 4 * HH, FP8), ("b64s", 4 * HH, FP8), ("k2t", 2 * N, BF16),
        ("kmt", 4 * HH, BF16), ("ih5", 4 * HH, BF16),
        ("tn", 2 * 1, F32),
    ]:
        dm[name] = nc.dram_tensor(name, [128, cols], dt, kind="ExternalInput")
    d_out = nc.dram_tensor("out", [1, BPC], F32, kind="ExternalOutput")

    with tile.TileContext(nc) as tc:
        with (
            tc.tile_pool(name="consts", bufs=1) as consts,
            tc.tile_pool(name="state", bufs=1) as state,
            tc.tile_pool(name="psum", bufs=1, space="PSUM") as psum,
        ):
            def loadc(name, chunks, eng):
                cols = dm[name].shape[1] // chunks
                t = consts.tile([128, chunks, cols], dm[name].dtype, tag=name)
                eng.dma_start(
                    out=t,
                    in_=dm[name].ap().rearrange("p (c x) -> p c x", c=chunks))
                return t

            # ---- DMA issue order chosen so the globally-serialized DMA
            # transfers run in deadline order:
            # trh, fc8, fs8, b64c, cv, b64s, w64, k2t, kmt, tn, ih5 ----
            tr = state.tile([128, 2, 128], BF16, tag="tr")
            nc.gpsimd.dma_start(
                out=tr, in_=d_trh.ap().rearrange("p (c x) -> p c x", c=2))
            fc8 = loadc("fc8", 2, nc.sync)       # SP#1
            fs8 = loadc("fs8", 2, nc.scalar)     # ACT#1
            b64c = loadc("b64c", 4, nc.sync)     # SP#2
            b64s = loadc("b64s", 4, nc.scalar)   # ACT#2
            w64 = loadc("w64", 4, nc.sync)       # SP#3
            k2t = loadc("k2t", 2, nc.scalar)     # ACT#3
            kmt = loadc("kmt", 4, nc.sync)       # SP#4
            ih5 = loadc("ih5", 4, nc.sync)       # SP#5
            tnt = loadc("tn", 2, nc.gpsimd)      # Pool#2

            ones = consts.tile([128, 1], BF16, tag="ones")
            nc.gpsimd.memset(ones, 1.0)
            # warm the ACT Square+Sqrt tables off the critical path
            # (~1.3us per set load). The warm input must be f32 — table
            # sets are input-dtype-specific and the real uses are f32.
            onesf = consts.tile([1, 1], F32, tag="onesf")
            nc.gpsimd.memset(onesf, 1.0)
            sqwarm = consts.tile([1, 2], F32, tag="sqwarm")
            nc.scalar.activation(
                out=sqwarm[:, 0:1], in_=onesf, func=ACTF.Square, scale=1.0)
            nc.scalar.activation(
                out=sqwarm[:, 1:2], in_=onesf, func=ACTF.Sqrt, scale=1.0)

            # ---- FFT of [t | r]: re/im = FC/FS @ tr  (free dim 128) ----
            re_ps = psum.tile([128, 4, 128], F32, tag="re")
            im_ps = psum.tile([128, 4, 128], F32, tag="im")
            for ps, w in ((re_ps, fc8), (im_ps, fs8)):
                for ot in range(4):
                    for kc in range(2):
                        nc.tensor.matmul(
                            ps[:, ot, :], w[:, kc, ot * 128:(ot + 1) * 128],
                            tr[:, kc, :], start=(kc == 0), stop=(kc == 1))
            ure = re_ps[:, :, 0:BPC]
            rre = re_ps[:, :, BPC:2 * BPC]
            uim = im_ps[:, :, 0:BPC]
            rim = im_ps[:, :, BPC:2 * BPC]

            # ---- Engine legality on real HW: GPSIMD (Pool) cannot
            # access PSUM and supports only plain TensorTensor ops;
            # DVE/ACT may read one PSUM operand; DVE supports
            # scalar_tensor_tensor. Both FFT outputs are copied wholesale
            # to SBUF with the 1/8 prescale folded in, so all products
            # are SBUF-only plain muls (no PSUM-reader serialization).
            # Products/squares carry exact power-of-2 scales: reb/imb
            # hold [U/8 | R/8], so X*Y products are /64 as the B64/W64
            # constants expect. ----
            reb = state.tile([128, 4, 128], F32, tag="reb")
            nc.vector.tensor_scalar_mul(reb, re_ps, 0.125)
            sqim = state.tile([128, 4, BPC], BF16, tag="sqim")
            nc.scalar.activation(out=sqim, in_=uim, func=ACTF.Square,
                                 scale=0.125)
            imb = state.tile([128, 4, 128], F32, tag="imb")
            nc.scalar.activation(out=imb, in_=im_ps, func=ACTF.Copy,
                                 scale=0.125)
            ureb = reb[:, :, 0:BPC]
            rreb = reb[:, :, BPC:2 * BPC]
            uimb = imb[:, :, 0:BPC]
            rimb = imb[:, :, BPC:2 * BPC]
            sqre = state.tile([128, 4, BPC], BF16, tag="sqre")
            nc.gpsimd.tensor_mul(sqre, ureb, ureb)
            t1 = state.tile([128, 4, BPC], BF16, tag="t1")
            t2 = state.tile([128, 4, BPC], BF16, tag="t2")
            t3 = state.tile([128, 4, BPC], BF16, tag="t3")
            t4n = state.tile([128, 4, BPC], BF16, tag="t4n")
            nc.gpsimd.tensor_mul(t1, ureb, rreb)
            lam64 = state.tile([128, 4, BPC], BF16, tag="lam64")
            nc.vector.scalar_tensor_tensor(
                out=lam64, in0=sqre, scalar=EPS / 64.0, in1=sqim,
                op0=AL.add, op1=AL.add)
            nc.gpsimd.tensor_mul(t2, uimb, rimb)
            nc.vector.scalar_tensor_tensor(
                out=t4n, in0=ureb, scalar=-1.0, in1=rimb, op0=AL.mult,
                op1=AL.mult)
            nc.gpsimd.tensor_mul(t3, uimb, rreb)

            # ---- lc = W64@lam64 first (lam64 ready early), then
            # bh = B64c@(t1+t2) + B64s@(t3+t4n) as one chain per gtile
            # (one open PSUM accumulation group per bank at a time);
            # late products (t2/t4n from Pool) ordered last per chain ----
            bha_ps = psum.tile([128, BPC], F32, tag="pb")
            bhb_ps = psum.tile([128, BPC], F32, tag="pg")
            lc_ps = psum.tile([128, 2, BPC], F32, tag="pa")
            for gt in range(2):
                for si, sq in enumerate((sqre, sqim)):
                    for kc in range(4):
                        nc.tensor.matmul(
                            lc_ps[:, gt, :],
                            w64[:, kc, gt * 128:(gt + 1) * 128],
                            sq[:, kc, :], start=(si == 0 and kc == 0),
                            stop=(si == 1 and kc == 3))
            terms = ((b64c, t1), (b64c, t2), (b64s, t4n), (b64s, t3))
            for ti, (w, t) in enumerate(terms):
                for gt, ps in ((0, bha_ps), (1, bhb_ps)):
                    for fc_ in range(4):
                        nc.tensor.matmul(
                            ps, w[:, fc_, gt * 128:(gt + 1) * 128],
                            t[:, fc_, :],
                            start=(ti == 0 and fc_ == 0),
                            stop=(ti == 3 and fc_ == 3))

            # ---- mu = 1/max(W@lam + cv, FLOOR) ----
            mu01 = state.tile([128, 2, BPC], F32, tag="mu01")
            mu = state.tile([128, 2, BPC], F32, tag="mu")
            nc.vector.tensor_scalar(
                out=mu01, in0=lc_ps, scalar1=EPS, scalar2=FLOOR,
                op0=AL.add, op1=AL.max)
            nc.vector.reciprocal(mu, mu01)

            # ---- x0 = K2(mu.bh); r0 = bh - 64 KM(lam64.x0); p0 = K2(mu.r0)

            def mm_k2(src_b, ptag):
                ps = psum.tile([128, 4, BPC], F32, tag=ptag)
                for ot in range(4):
                    for gc in range(2):
                        nc.tensor.matmul(
                            ps[:, ot, :],
                            k2t[:, gc, ot * 128:(ot + 1) * 128],
                            src_b[:, gc, :], start=(gc == 0), stop=(gc == 1))
                return ps

            def mm_km(src_b, ptag):
                ps = psum.tile([128, 2, BPC], F32, tag=ptag)
                for gt in range(2):
                    for fc_ in range(4):
                        nc.tensor.matmul(
                            ps[:, gt, :],
                            kmt[:, fc_, gt * 128:(gt + 1) * 128],
                            src_b[:, fc_, :], start=(fc_ == 0),
                            stop=(fc_ == 3))
                return ps

            sh0 = state.tile([128, 2, BPC], BF16, tag="sh0")
            nc.vector.tensor_mul(sh0[:, 0, :], mu[:, 0, :], bha_ps)
            nc.vector.tensor_mul(sh0[:, 1, :], mu[:, 1, :], bhb_ps)
            x0_ps = mm_k2(sh0, "pc")

            th0 = state.tile([128, 4, BPC], BF16, tag="th0")
            nc.vector.tensor_mul(th0, lam64, x0_ps)
            bhs = state.tile([128, 2, BPC], F32, tag="bhs")
            nc.scalar.copy(bhs[:, 0, :], bha_ps)
            nc.scalar.copy(bhs[:, 1, :], bhb_ps)
            x0s = state.tile([128, 4, BPC], F32, tag="x0s")
            nc.scalar.copy(x0s, x0_ps)
            g0_ps = mm_km(th0, "pa")
            rh = state.tile([128, 2, BPC], F32, tag="rh")
            nc.vector.scalar_tensor_tensor(
                out=rh, in0=g0_ps, scalar=-64.0, in1=bhs, op0=AL.mult,
                op1=AL.add)
            sh = state.tile([128, 2, BPC], BF16, tag="sh")
            nc.vector.tensor_mul(sh, mu, rh)
            p0_ps = mm_k2(sh, "pd")

            # ---- one full iteration (constant scalars) ----
            th = state.tile([128, 4, BPC], BF16, tag="th")
            nc.vector.tensor_mul(th, lam64, p0_ps)
            p0s = state.tile([128, 4, BPC], F32, tag="p0s")
            nc.scalar.copy(p0s, p0_ps)
            gh_ps = mm_km(th, "pa")
            rh2 = state.tile([128, 2, BPC], F32, tag="rh2")
            nc.vector.scalar_tensor_tensor(
                out=rh2, in0=gh_ps, scalar=-64.0 * AL0, in1=rh, op0=AL.mult,
                op1=AL.add)
            sh2 = state.tile([128, 2, BPC], BF16, tag="sh2")
            nc.vector.tensor_mul(sh2, mu, rh2)
            k2_ps = mm_k2(sh2, "pf")

            # xh2 = x0 + (a0 + a1 b1) p0 on Pool, off the critical path
            xh2 = state.tile([128, 4, BPC], F32, tag="xh2")
            nc.gpsimd.tensor_scalar_mul(xh2, p0s, XS)
            nc.gpsimd.tensor_add(xh2, xh2, x0s)

            u = state.tile([128, 4, BPC], BF16, tag="u")
            nc.vector.scalar_tensor_tensor(
                out=u, in0=k2_ps, scalar=AL1, in1=xh2, op0=AL.mult,
                op1=AL.add)

            # ---- finale: vN = IH5^T u (n-partition layout), ratios ----
            vN_ps = psum.tile([128, 2, BPC], F32, tag="im")
            for nt in range(2):
                for fc_ in range(4):
                    nc.tensor.matmul(
                        vN_ps[:, nt, :],
                        ih5[:, fc_, nt * 128:(nt + 1) * 128], u[:, fc_, :],
                        start=(fc_ == 0), stop=(fc_ == 3))
            sqD = state.tile([128, 2, BPC], BF16, tag="sqD")
            nc.scalar.activation(out=sqD, in_=vN_ps, func=ACTF.Square,
                                 scale=1.0)
            # sqT = (T.v)^2 = T^2 * sqD — stays on DVE, reads SBUF
            sqT = state.tile([128, 2, BPC], BF16, tag="sqT")
            for nt in range(2):
                nc.vector.tensor_scalar(
                    out=sqT[:, nt, :], in0=sqD[:, nt, :],
                    scalar1=tnt[:, nt, :], scalar2=None, op0=AL.mult)
            den2_ps = psum.tile([1, BPC], F32, tag="pc")
            num2_ps = psum.tile([1, BPC], F32, tag="pa")
            for nt in range(2):
                nc.tensor.matmul(den2_ps, ones, sqD[:, nt, :],
                                 start=(nt == 0), stop=(nt == 1))
            for nt in range(2):
                nc.tensor.matmul(num2_ps, ones, sqT[:, nt, :],
                                 start=(nt == 0), stop=(nt == 1))
            iden = state.tile([1, BPC], F32, tag="iden")
            nc.vector.reciprocal(iden, den2_ps)
            rat = state.tile([1, BPC], F32, tag="rat")
            nc.vector.tensor_mul(rat, num2_ps, iden)
            srat = state.tile([1, BPC], F32, tag="srat")
            nc.scalar.activation(out=srat, in_=rat, func=ACTF.Sqrt,
                                 scale=0.25)
            nc.sync.dma_start(out=d_out.ap(), in_=srat)

    nc.finalize()
    return nc


def _pack_inputs(recon, target):
    """Per-core [128, 256] bf16 DMA payloads: inputs prescaled by 1/8
    (exact) to match the x8 FFT weight prescale; partition p row c holds
    [target[:, c*128+p] | recon[:, c*128+p]]."""
    bf16 = _bf16np()
    outs = []
    for c in range(NCORES):
        sl = slice(c * BPC, (c + 1) * BPC)
        tt = (target[sl].astype(np.float32) * 0.125).astype(bf16)
        rr = (recon[sl].astype(np.float32) * 0.125).astype(bf16)
        tr3 = np.empty((128, 2, 2 * BPC), dtype=bf16)
        for kc in range(2):
            tr3[:, kc, 0:BPC] = tt[:, kc * 128:(kc + 1) * 128].T
            tr3[:, kc, BPC:2 * BPC] = rr[:, kc * 128:(kc + 1) * 128].T
        outs.append(np.ascontiguousarray(tr3.reshape(128, 2 * 128)))
    return outs


def kernel(recon: np.ndarray, target: np.ndarray) -> np.ndarray:
    from concourse.bass_utils import run_bass_kernel_spmd

    consts = _host_consts()
    nc = _program()

    trhs = _pack_inputs(recon, target)
    in_maps = []
    for c in range(NCORES):
        m = dict(consts)
        m["trh"] = trhs[c]
        in_maps.append(m)

    res = run_bass_kernel_spmd(nc, in_maps, core_ids=list(range(NCORES)))
    kernel._last_results = res  # for test.py introspection (profiling)
    total = 0.0
    for c in range(NCORES):
        total += float(res.results[c]["out"].astype(np.float64).sum())
    return np.float32(total)


# revision 7
# speedup vs baseline: 1.0585x; 1.0044x over previous
"""AWLoss1D batched-Toeplitz-solve loss on 8 Trainium2 NeuronCores.

Math (per batch row b of 512):
  D_b = (511x256) Toeplitz of target_b;  A_b = D^T D + eps*I;
  v_b = A_b^{-1} (D^T pad(recon_b));  loss = sum_b 0.5*||T.v||/||v||.

Device algorithm (64 systems per core, pure data parallel):
  * A_b embeds in the 512-circulant with eigenvalues lam_b =
    |FFT_512(target_b zero-padded)|^2; lam symmetric => diagonalized by the
    real 512-point Hartley transform H5 shared by all batches, so batched
    matvecs are plain PE matmuls with batch on the free dim.
  * Two-step CHEBYSHEV-style iteration with data-independent scalars
    (alpha0, alpha1, beta1) tuned numerically on the reference model —
    no dot products, no reciprocals, no per-batch scalar broadcasts.
    All spectral state: xh/ph 512-spectra, rh 256-Hartley spectra;
    KM = (1/512) H2 H5^T and K2 = (1/256) H5 H2^T map between domains.
  * Preconditioner: blended Chan/Strang circulant, spectrum mu from
    lam via a host-precomputed 256x512 map W.
  * Matmul dtypes: moving operands bf16 (1 PE cycle/row vs 4 for f32);
    setup-only weights (FFT/B/W) fp8 with power-of-2 prescales folded
    into host constants and immediates; KM/K2/IH5 weights bf16.
    lam64 = lam/64 is the only stored spectrum (the 64 folds into the
    residual-update immediates).
  * RHS spectrum directly: bh = B64c@(Zre/64) + B64s@(Zimn/64) where
    Z = conj(FFT(target)).FFT(recon), pad-127 shift folded into B on host.
  * Finale: v^T = IH5^T (x0 + (a0+a1*b1) p0 + a1 k2) as one fused update;
    per-batch ||Tv||^2/||v||^2 via ACT Square + ones-matmul partition
    reductions; per-batch 0.5*sqrt on ACT; host sums the 8x64 partials.
"""
import functools

import numpy as np

B, HH, N, NCORES = 512, 256, 512, 8
BPC = B // NCORES  # 64 batches per core
EPS = 1e-4
FLOOR = 0.1

# Chebyshev-style scalars, tuned on the f64 reference with the exact
# device rounding model (robust to +-2% perturbation).
AL0 = 0.18383249176451916
AL1 = 0.3939614782927838
BE1 = 0.2285507684190372
XS = AL0 + AL1 * BE1


def _bf16np():
    import ml_dtypes
    return ml_dtypes.bfloat16


def _fp8np():
    import ml_dtypes
    return ml_dtypes.float8_e4m3


@functools.lru_cache(maxsize=1)
def _host_consts():
    """Constant matrices in f64, quantized and pre-swizzled to the
    [128, chunks*cols] per-partition-contiguous DMA layout."""
    bf16 = _bf16np()
    fp8 = _fp8np()

    n5 = np.arange(N)
    n2 = np.arange(HH)
    ang5 = 2.0 * np.pi * np.outer(n5, n5) / N
    cas5 = np.cos(ang5) + np.sin(ang5)
    ang2 = 2.0 * np.pi * np.outer(n2, n2) / HH
    cas2 = np.cos(ang2) + np.sin(ang2)
    H5 = cas5[:, :HH]                                   # [512 f, 256 n]
    H2 = cas2                                           # [256 g, 256 n]

    KMT = ((H2 @ H5.T) / N).T.copy()                    # lhsT [512 f, 256 g]
    K2T = ((H5 @ H2.T) / HH).T.copy()                   # lhsT [256 g, 512 f]
    IH5 = (cas5 / N)[:, :HH].copy()                     # lhsT [512 f, 256 n]
    FCT = (8.0 * np.cos(ang5))[:, :HH].T.copy()         # lhsT [256 n, 512 f]
    FST = (-8.0 * np.sin(ang5))[:, :HH].T.copy()
    angb = 2.0 * np.pi * np.outer(n5, n2 - 127.0) / N   # [f, j]
    BCm = np.cos(angb) / N
    BSm = np.sin(angb) / N
    BCHT = (64.0 * (H2 @ BCm.T)).T.copy()               # lhsT [512 f, 256 g]
    BSHT = (64.0 * (H2 @ BSm.T)).T.copy()
    # preconditioner spectrum map: 0.35/0.65 Chan/Strang circulant blend
    RHO = np.cos(2.0 * np.pi * np.outer(n2, n5) / N) / N
    CW_chan = np.zeros((HH, HH))
    CW_chan[n2, n2] += (HH - n2) / HH
    CW_chan[n2, (HH - n2) % HH] += n2 / HH
    CW_str = np.zeros((HH, HH))
    CW_str[n2, n2] += 1.0
    CW_str[n2[1:], (HH - n2[1:]) % HH] += 1.0
    CW = 0.35 * CW_chan + 0.65 * CW_str
    DCT = np.cos(2.0 * np.pi * np.outer(n2, n2) / HH)
    W = DCT @ CW @ RHO                                  # [256 g, 512 f]
    WT = (64.0 * W).T.copy()                            # lhsT [512 f, 256 g]
    cv = (EPS * (1.0 - W.sum(axis=1)))[:, None].copy()  # [256 g, 1]

    x = np.linspace(-10.0, 10.0, HH)
    dx = (x[-1] - x[0]) / (HH - 1)
    dispx = (HH % 2 - 1) / 2.0
    g = -np.exp(-((x - dx * dispx) ** 2) / 2.0)
    g = g + np.max(np.abs(g))
    Tw = ((g / np.max(np.abs(g))) ** 2)[:, None].copy()  # T^2, [256 n, 1]

    def swz(a, dt):
        """[C*128, X] lhsT -> [128, C*X] with partition rows contiguous."""
        a = np.asarray(a, dtype=np.float32)
        c = a.shape[0] // 128
        return np.ascontiguousarray(
            a.reshape(c, 128, a.shape[1]).transpose(1, 0, 2).reshape(
                128, c * a.shape[1])).astype(dt)

    return {
        "fc8": swz(FCT, fp8), "fs8": swz(FST, fp8),
        "w64": swz(WT, fp8), "b64c": swz(BCHT, fp8), "b64s": swz(BSHT, fp8),
        "k2t": swz(K2T, bf16), "kmt": swz(KMT, bf16), "ih5": swz(IH5, bf16),
        "tn": swz(Tw, np.float32),
    }


@functools.lru_cache(maxsize=1)
def _program():
    import concourse.bacc as bacc
    import concourse.mybir as mybir
    import concourse.tile as tile

    F32 = mybir.dt.float32
    BF16 = mybir.dt.bfloat16
    FP8 = mybir.dt.float8e4
    AL = mybir.AluOpType
    ACTF = mybir.ActivationFunctionType

    nc = bacc.Bacc(target_bir_lowering=False)

    d_trh = nc.dram_tensor("trh", [128, 2 * 128], BF16, kind="ExternalInput")
    dm = {}
    for name, cols, dt in [
        ("fc8", 2 * N, FP8), ("fs8", 2 * N, FP8), ("w64", 4 * HH, FP8),
        ("b64c", 4 * HH, FP8), ("b64s", 4 * HH, FP8), ("k2t", 2 * N, BF16),
        ("kmt", 4 * HH, BF16), ("ih5", 4 * HH, BF16),
        ("tn", 2 * 1, F32),
    ]:
        dm[name] = nc.dram_tensor(name, [128, cols], dt, kind="ExternalInput")
    d_out = nc.dram_tensor("out", [1, BPC], F32, kind="ExternalOutput")

    with tile.TileContext(nc) as tc:
        with (
            tc.tile_pool(name="consts", bufs=1) as consts,
            tc.tile_pool(name="state", bufs=1) as state,
            tc.tile_pool(name="psum", bufs=1, space="PSUM") as psum,
        ):
            def loadc(name, chunks, eng):
                cols = dm[name].shape[1] // chunks
                t = consts.tile([128, chunks, cols], dm[name].dtype, tag=name)
                eng.dma_start(
                    out=t,
                    in_=dm[name].ap().rearrange("p (c x) -> p c x", c=chunks))
                return t

            # ---- DMA issue order chosen so the globally-serialized DMA
            # transfers run in deadline order:
            # trh, fc8, fs8, b64c, cv, b64s, w64, k2t, kmt, tn, ih5 ----
            tr = state.tile([128, 2, 128], BF16, tag="tr")
            nc.gpsimd.dma_start(
                out=tr, in_=d_trh.ap().rearrange("p (c x) -> p c x", c=2))
            fc8 = loadc("fc8", 2, nc.sync)       # SP#1
            fs8 = loadc("fs8", 2, nc.sync)       # SP#2
            b64c = loadc("b64c", 4, nc.sync)     # SP#3
            b64s = loadc("b64s", 4, nc.sync)     # SP#4
            w64 = loadc("w64", 4, nc.sync)       # SP#5
            k2t = loadc("k2t", 2, nc.sync)       # SP#6
            kmt = loadc("kmt", 4, nc.sync)       # SP#7
            ih5 = loadc("ih5", 4, nc.sync)       # SP#8
            tnt = loadc("tn", 2, nc.gpsimd)      # Pool#2

            ones = consts.tile([128, 1], BF16, tag="ones")
            nc.gpsimd.memset(ones, 1.0)
            # warm the ACT Square+Sqrt tables off the critical path
            # (~1.3us per set load). The warm input must be f32 — table
            # sets are input-dtype-specific and the real uses are f32.
            onesf = consts.tile([1, 1], F32, tag="onesf")
            nc.gpsimd.memset(onesf, 1.0)
            sqwarm = consts.tile([1, 2], F32, tag="sqwarm")
            nc.scalar.activation(
                out=sqwarm[:, 1:2], in_=onesf, func=ACTF.Sqrt, scale=1.0)

            # ---- FFT of [t | r]: re/im = FC/FS @ tr  (free dim 128) ----
            re_ps = psum.tile([128, 4, 128], F32, tag="re")
            im_ps = psum.tile([128, 4, 128], F32, tag="im")
            for ps, w in ((re_ps, fc8), (im_ps, fs8)):
                for ot in range(4):
                    for kc in range(2):
                        nc.tensor.matmul(
                            ps[:, ot, :], w[:, kc, ot * 128:(ot + 1) * 128],
                            tr[:, kc, :], start=(kc == 0), stop=(kc == 1))
            ure = re_ps[:, :, 0:BPC]
            rre = re_ps[:, :, BPC:2 * BPC]
            uim = im_ps[:, :, 0:BPC]
            rim = im_ps[:, :, BPC:2 * BPC]

            # ---- Engine legality on real HW: GPSIMD (Pool) cannot
            # access PSUM and supports only plain TensorTensor ops;
            # DVE/ACT may read one PSUM operand; DVE supports
            # scalar_tensor_tensor. Both FFT outputs are copied wholesale
            # to SBUF with the 1/8 prescale folded in, so all products
            # are SBUF-only plain muls (no PSUM-reader serialization).
            # Products/squares carry exact power-of-2 scales: reb/imb
            # hold [U/8 | R/8], so X*Y products are /64 as the B64/W64
            # constants expect. ----
            reb = state.tile([128, 4, 128], F32, tag="reb")
            nc.vector.tensor_scalar_mul(reb, re_ps, 0.125)
            sqim = state.tile([128, 4, BPC], BF16, tag="sqim")
            nc.scalar.activation(out=sqim, in_=uim, func=ACTF.Square,
                                 scale=0.125)
            imb = state.tile([128, 4, 128], F32, tag="imb")
            nc.scalar.activation(out=imb, in_=im_ps, func=ACTF.Copy,
                                 scale=0.125)
            ureb = reb[:, :, 0:BPC]
            rreb = reb[:, :, BPC:2 * BPC]
            uimb = imb[:, :, 0:BPC]
            rimb = imb[:, :, BPC:2 * BPC]
            sqre = state.tile([128, 4, BPC], BF16, tag="sqre")
            nc.gpsimd.tensor_mul(sqre, ureb, ureb)
            t1 = state.tile([128, 4, BPC], BF16, tag="t1")
            t2 = state.tile([128, 4, BPC], BF16, tag="t2")
            t3 = state.tile([128, 4, BPC], BF16, tag="t3")
            t4n = state.tile([128, 4, BPC], BF16, tag="t4n")
            nc.gpsimd.tensor_mul(t1, ureb, rreb)
            lam64 = state.tile([128, 4, BPC], BF16, tag="lam64")
            nc.vector.scalar_tensor_tensor(
                out=lam64, in0=sqre, scalar=EPS / 64.0, in1=sqim,
                op0=AL.add, op1=AL.add)
            nc.gpsimd.tensor_mul(t2, uimb, rimb)
            nc.vector.scalar_tensor_tensor(
                out=t4n, in0=ureb, scalar=-1.0, in1=rimb, op0=AL.mult,
                op1=AL.mult)
            nc.gpsimd.tensor_mul(t3, uimb, rreb)

            # ---- lc = W64@lam64 first (lam64 ready early), then
            # bh = B64c@(t1+t2) + B64s@(t3+t4n) as one chain per gtile
            # (one open PSUM accumulation group per bank at a time);
            # late products (t2/t4n from Pool) ordered last per chain ----
            bha_ps = psum.tile([128, BPC], F32, tag="pb")
            bhb_ps = psum.tile([128, BPC], F32, tag="pg")
            lc_ps = psum.tile([128, 2, BPC], F32, tag="pa")
            for gt in range(2):
                for si, sq in enumerate((sqre, sqim)):
                    for kc in range(4):
                        nc.tensor.matmul(
                            lc_ps[:, gt, :],
                            w64[:, kc, gt * 128:(gt + 1) * 128],
                            sq[:, kc, :], start=(si == 0 and kc == 0),
                            stop=(si == 1 and kc == 3))
            terms = ((b64c, t1), (b64c, t2), (b64s, t4n), (b64s, t3))
            for ti, (w, t) in enumerate(terms):
                for gt, ps in ((0, bha_ps), (1, bhb_ps)):
                    for fc_ in range(4):
                        nc.tensor.matmul(
                            ps, w[:, fc_, gt * 128:(gt + 1) * 128],
                            t[:, fc_, :],
                            start=(ti == 0 and fc_ == 0),
                            stop=(ti == 3 and fc_ == 3))

            # ---- mu = 1/max(W@lam + cv, FLOOR) ----
            mu01 = state.tile([128, 2, BPC], F32, tag="mu01")
            mu = state.tile([128, 2, BPC], F32, tag="mu")
            nc.vector.tensor_scalar(
                out=mu01, in0=lc_ps, scalar1=EPS, scalar2=FLOOR,
                op0=AL.add, op1=AL.max)
            nc.vector.reciprocal(mu, mu01)

            # ---- x0 = K2(mu.bh); r0 = bh - 64 KM(lam64.x0); p0 = K2(mu.r0)

            def mm_k2(src_b, ptag):
                ps = psum.tile([128, 4, BPC], F32, tag=ptag)
                for ot in range(4):
                    for gc in range(2):
                        nc.tensor.matmul(
                            ps[:, ot, :],
                            k2t[:, gc, ot * 128:(ot + 1) * 128],
                            src_b[:, gc, :], start=(gc == 0), stop=(gc == 1))
                return ps

            def mm_km(src_b, ptag):
                ps = psum.tile([128, 2, BPC], F32, tag=ptag)
                for gt in range(2):
                    for fc_ in range(4):
                        nc.tensor.matmul(
                            ps[:, gt, :],
                            kmt[:, fc_, gt * 128:(gt + 1) * 128],
                            src_b[:, fc_, :], start=(fc_ == 0),
                            stop=(fc_ == 3))
                return ps

            sh0 = state.tile([128, 2, BPC], BF16, tag="sh0")
            nc.vector.tensor_mul(sh0[:, 0, :], mu[:, 0, :], bha_ps)
            nc.vector.tensor_mul(sh0[:, 1, :], mu[:, 1, :], bhb_ps)
            x0_ps = mm_k2(sh0, "pc")

            th0 = state.tile([128, 4, BPC], BF16, tag="th0")
            nc.vector.tensor_mul(th0, lam64, x0_ps)
            bhs = state.tile([128, 2, BPC], F32, tag="bhs")
            nc.scalar.copy(bhs[:, 0, :], bha_ps)
            nc.scalar.copy(bhs[:, 1, :], bhb_ps)
            x0s = state.tile([128, 4, BPC], F32, tag="x0s")
            nc.scalar.copy(x0s, x0_ps)
            g0_ps = mm_km(th0, "pa")
            rh = state.tile([128, 2, BPC], F32, tag="rh")
            nc.vector.scalar_tensor_tensor(
                out=rh, in0=g0_ps, scalar=-64.0, in1=bhs, op0=AL.mult,
                op1=AL.add)
            sh = state.tile([128, 2, BPC], BF16, tag="sh")
            nc.vector.tensor_mul(sh, mu, rh)
            p0_ps = mm_k2(sh, "pd")

            # ---- one full iteration (constant scalars) ----
            th = state.tile([128, 4, BPC], BF16, tag="th")
            nc.vector.tensor_mul(th, lam64, p0_ps)
            p0s = state.tile([128, 4, BPC], F32, tag="p0s")
            nc.scalar.copy(p0s, p0_ps)
            gh_ps = mm_km(th, "pa")
            rh2 = state.tile([128, 2, BPC], F32, tag="rh2")
            nc.vector.scalar_tensor_tensor(
                out=rh2, in0=gh_ps, scalar=-64.0 * AL0, in1=rh, op0=AL.mult,
                op1=AL.add)
            sh2 = state.tile([128, 2, BPC], BF16, tag="sh2")
            nc.vector.tensor_mul(sh2, mu, rh2)
            k2_ps = mm_k2(sh2, "pf")

            # xh2 = x0 + (a0 + a1 b1) p0 on Pool, off the critical path
            xh2 = state.tile([128, 4, BPC], F32, tag="xh2")
            nc.gpsimd.tensor_scalar_mul(xh2, p0s, XS)
            nc.gpsimd.tensor_add(xh2, xh2, x0s)

            u = state.tile([128, 4, BPC], BF16, tag="u")
            nc.vector.scalar_tensor_tensor(
                out=u, in0=k2_ps, scalar=AL1, in1=xh2, op0=AL.mult,
                op1=AL.add)

            # ---- finale: vN = IH5^T u (n-partition layout), ratios ----
            vN_ps = psum.tile([128, 2, BPC], F32, tag="im")
            for nt in range(2):
                for fc_ in range(4):
                    nc.tensor.matmul(
                        vN_ps[:, nt, :],
                        ih5[:, fc_, nt * 128:(nt + 1) * 128], u[:, fc_, :],
                        start=(fc_ == 0), stop=(fc_ == 3))
            sqD = state.tile([128, 2, BPC], BF16, tag="sqD")
            nc.scalar.activation(out=sqD, in_=vN_ps, func=ACTF.Square,
                                 scale=1.0)
            # sqT = (T.v)^2 = T^2 * sqD — stays on DVE, reads SBUF
            sqT = state.tile([128, 2, BPC], BF16, tag="sqT")
            for nt in range(2):
                nc.vector.tensor_scalar(
                    out=sqT[:, nt, :], in0=sqD[:, nt, :],
                    scalar1=tnt[:, nt, :], scalar2=None, op0=AL.mult)
            den2_ps = psum.tile([1, BPC], F32, tag="pc")
            num2_ps = psum.tile([1, BPC], F32, tag="pa")
            for nt in range(2):
                nc.tensor.matmul(den2_ps, ones, sqD[:, nt, :],
                                 start=(nt == 0), stop=(nt == 1))
            for nt in range(2):
                nc.tensor.matmul(num2_ps, ones, sqT[:, nt, :],
                                 start=(nt == 0), stop=(nt == 1))
            iden = state.tile([1, BPC], F32, tag="iden")
            nc.vector.reciprocal(iden, den2_ps)
            rat = state.tile([1, BPC], F32, tag="rat")
            nc.vector.tensor_mul(rat, num2_ps, iden)
            srat = state.tile([1, BPC], F32, tag="srat")
            nc.scalar.activation(out=srat, in_=rat, func=ACTF.Sqrt,
                                 scale=0.25)
            nc.sync.dma_start(out=d_out.ap(), in_=srat)

    nc.finalize()
    return nc


def _pack_inputs(recon, target):
    """Per-core [128, 256] bf16 DMA payloads: inputs prescaled by 1/8
    (exact) to match the x8 FFT weight prescale; partition p row c holds
    [target[:, c*128+p] | recon[:, c*128+p]]."""
    bf16 = _bf16np()
    outs = []
    for c in range(NCORES):
        sl = slice(c * BPC, (c + 1) * BPC)
        tt = (target[sl].astype(np.float32) * 0.125).astype(bf16)
        rr = (recon[sl].astype(np.float32) * 0.125).astype(bf16)
        tr3 = np.empty((128, 2, 2 * BPC), dtype=bf16)
        for kc in range(2):
            tr3[:, kc, 0:BPC] = tt[:, kc * 128:(kc + 1) * 128].T
            tr3[:, kc, BPC:2 * BPC] = rr[:, kc * 128:(kc + 1) * 128].T
        outs.append(np.ascontiguousarray(tr3.reshape(128, 2 * 128)))
    return outs


def kernel(recon: np.ndarray, target: np.ndarray) -> np.ndarray:
    from concourse.bass_utils import run_bass_kernel_spmd

    consts = _host_consts()
    nc = _program()

    trhs = _pack_inputs(recon, target)
    in_maps = []
    for c in range(NCORES):
        m = dict(consts)
        m["trh"] = trhs[c]
        in_maps.append(m)

    res = run_bass_kernel_spmd(nc, in_maps, core_ids=list(range(NCORES)))
    kernel._last_results = res  # for test.py introspection (profiling)
    total = 0.0
    for c in range(NCORES):
        total += float(res.results[c]["out"].astype(np.float64).sum())
    return np.float32(total)


# revision 8
# speedup vs baseline: 1.0605x; 1.0019x over previous
"""AWLoss1D batched-Toeplitz-solve loss on 8 Trainium2 NeuronCores.

Math (per batch row b of 512):
  D_b = (511x256) Toeplitz of target_b;  A_b = D^T D + eps*I;
  v_b = A_b^{-1} (D^T pad(recon_b));  loss = sum_b 0.5*||T.v||/||v||.

Device algorithm (64 systems per core, pure data parallel):
  * A_b embeds in the 512-circulant with eigenvalues lam_b =
    |FFT_512(target_b zero-padded)|^2; lam symmetric => diagonalized by the
    real 512-point Hartley transform H5 shared by all batches, so batched
    matvecs are plain PE matmuls with batch on the free dim.
  * Two-step CHEBYSHEV-style iteration with data-independent scalars
    (alpha0, alpha1, beta1) tuned numerically on the reference model —
    no dot products, no reciprocals, no per-batch scalar broadcasts.
    All spectral state: xh/ph 512-spectra, rh 256-Hartley spectra;
    KM = (1/512) H2 H5^T and K2 = (1/256) H5 H2^T map between domains.
  * Preconditioner: blended Chan/Strang circulant, spectrum mu from
    lam via a host-precomputed 256x512 map W.
  * Matmul dtypes: moving operands bf16 (1 PE cycle/row vs 4 for f32);
    setup-only weights (FFT/B/W) fp8 with power-of-2 prescales folded
    into host constants and immediates; KM/K2/IH5 weights bf16.
    lam64 = lam/64 is the only stored spectrum (the 64 folds into the
    residual-update immediates).
  * RHS spectrum directly: bh = B64c@(Zre/64) + B64s@(Zimn/64) where
    Z = conj(FFT(target)).FFT(recon), pad-127 shift folded into B on host.
  * Finale: v^T = IH5^T (x0 + (a0+a1*b1) p0 + a1 k2) as one fused update;
    per-batch ||Tv||^2/||v||^2 via ACT Square + ones-matmul partition
    reductions; per-batch 0.5*sqrt on ACT; host sums the 8x64 partials.
"""
import functools

import numpy as np

B, HH, N, NCORES = 512, 256, 512, 8
BPC = B // NCORES  # 64 batches per core
EPS = 1e-4
FLOOR = 0.1

# Chebyshev-style scalars, tuned on the f64 reference with the exact
# device rounding model (robust to +-2% perturbation).
AL0 = 0.18383249176451916
AL1 = 0.3939614782927838
BE1 = 0.2285507684190372
XS = AL0 + AL1 * BE1


def _bf16np():
    import ml_dtypes
    return ml_dtypes.bfloat16


def _fp8np():
    import ml_dtypes
    return ml_dtypes.float8_e4m3


@functools.lru_cache(maxsize=1)
def _host_consts():
    """Constant matrices in f64, quantized and pre-swizzled to the
    [128, chunks*cols] per-partition-contiguous DMA layout."""
    bf16 = _bf16np()
    fp8 = _fp8np()

    n5 = np.arange(N)
    n2 = np.arange(HH)
    ang5 = 2.0 * np.pi * np.outer(n5, n5) / N
    cas5 = np.cos(ang5) + np.sin(ang5)
    ang2 = 2.0 * np.pi * np.outer(n2, n2) / HH
    cas2 = np.cos(ang2) + np.sin(ang2)
    H5 = cas5[:, :HH]                                   # [512 f, 256 n]
    H2 = cas2                                           # [256 g, 256 n]

    KMT = ((H2 @ H5.T) / N).T.copy()                    # lhsT [512 f, 256 g]
    K2T = ((H5 @ H2.T) / HH).T.copy()                   # lhsT [256 g, 512 f]
    IH5 = (cas5 / N)[:, :HH].copy()                     # lhsT [512 f, 256 n]
    FCT = (8.0 * np.cos(ang5))[:, :HH].T.copy()         # lhsT [256 n, 512 f]
    FST = (-8.0 * np.sin(ang5))[:, :HH].T.copy()
    angb = 2.0 * np.pi * np.outer(n5, n2 - 127.0) / N   # [f, j]
    BCm = np.cos(angb) / N
    BSm = np.sin(angb) / N
    BCHT = (64.0 * (H2 @ BCm.T)).T.copy()               # lhsT [512 f, 256 g]
    BSHT = (64.0 * (H2 @ BSm.T)).T.copy()
    # preconditioner spectrum map: 0.35/0.65 Chan/Strang circulant blend
    RHO = np.cos(2.0 * np.pi * np.outer(n2, n5) / N) / N
    CW_chan = np.zeros((HH, HH))
    CW_chan[n2, n2] += (HH - n2) / HH
    CW_chan[n2, (HH - n2) % HH] += n2 / HH
    CW_str = np.zeros((HH, HH))
    CW_str[n2, n2] += 1.0
    CW_str[n2[1:], (HH - n2[1:]) % HH] += 1.0
    CW = 0.35 * CW_chan + 0.65 * CW_str
    DCT = np.cos(2.0 * np.pi * np.outer(n2, n2) / HH)
    W = DCT @ CW @ RHO                                  # [256 g, 512 f]
    WT = (64.0 * W).T.copy()                            # lhsT [512 f, 256 g]
    cv = (EPS * (1.0 - W.sum(axis=1)))[:, None].copy()  # [256 g, 1]

    x = np.linspace(-10.0, 10.0, HH)
    dx = (x[-1] - x[0]) / (HH - 1)
    dispx = (HH % 2 - 1) / 2.0
    g = -np.exp(-((x - dx * dispx) ** 2) / 2.0)
    g = g + np.max(np.abs(g))
    Tw = ((g / np.max(np.abs(g))) ** 2)[:, None].copy()  # T^2, [256 n, 1]

    def swz(a, dt):
        """[C*128, X] lhsT -> [128, C*X] with partition rows contiguous."""
        a = np.asarray(a, dtype=np.float32)
        c = a.shape[0] // 128
        return np.ascontiguousarray(
            a.reshape(c, 128, a.shape[1]).transpose(1, 0, 2).reshape(
                128, c * a.shape[1])).astype(dt)

    return {
        "fc8": swz(FCT, fp8), "fs8": swz(FST, fp8),
        "w64": swz(WT, fp8), "b64c": swz(BCHT, fp8), "b64s": swz(BSHT, fp8),
        "k2t": swz(K2T, bf16), "kmt": swz(KMT, bf16), "ih5": swz(IH5, bf16),
        "tn": swz(Tw, np.float32),
    }


@functools.lru_cache(maxsize=1)
def _program():
    import concourse.bacc as bacc
    import concourse.mybir as mybir
    import concourse.tile as tile

    F32 = mybir.dt.float32
    BF16 = mybir.dt.bfloat16
    FP8 = mybir.dt.float8e4
    AL = mybir.AluOpType
    ACTF = mybir.ActivationFunctionType

    nc = bacc.Bacc(target_bir_lowering=False)

    d_trh = nc.dram_tensor("trh", [128, 2 * 128], BF16, kind="ExternalInput")
    dm = {}
    for name, cols, dt in [
        ("fc8", 2 * N, FP8), ("fs8", 2 * N, FP8), ("w64", 4 * HH, FP8),
        ("b64c", 4 * HH, FP8), ("b64s", 4 * HH, FP8), ("k2t", 2 * N, BF16),
        ("kmt", 4 * HH, BF16), ("ih5", 4 * HH, BF16),
        ("tn", 2 * 1, F32),
    ]:
        dm[name] = nc.dram_tensor(name, [128, cols], dt, kind="ExternalInput")
    d_out = nc.dram_tensor("out", [1, BPC], F32, kind="ExternalOutput")

    with tile.TileContext(nc) as tc:
        with (
            tc.tile_pool(name="consts", bufs=1) as consts,
            tc.tile_pool(name="state", bufs=1) as state,
            tc.tile_pool(name="psum", bufs=1, space="PSUM") as psum,
        ):
            def loadc(name, chunks, eng):
                cols = dm[name].shape[1] // chunks
                t = consts.tile([128, chunks, cols], dm[name].dtype, tag=name)
                eng.dma_start(
                    out=t,
                    in_=dm[name].ap().rearrange("p (c x) -> p c x", c=chunks))
                return t

            # ---- DMA issue order chosen so the globally-serialized DMA
            # transfers run in deadline order:
            # trh, fc8, fs8, b64c, cv, b64s, w64, k2t, kmt, tn, ih5 ----
            tr = state.tile([128, 2, 128], BF16, tag="tr")
            nc.gpsimd.dma_start(
                out=tr, in_=d_trh.ap().rearrange("p (c x) -> p c x", c=2))
            fc8 = loadc("fc8", 2, nc.sync)       # SP#1
            fs8 = loadc("fs8", 2, nc.sync)       # SP#2
            b64c = loadc("b64c", 4, nc.sync)     # SP#3
            b64s = loadc("b64s", 4, nc.sync)     # SP#4
            w64 = loadc("w64", 4, nc.sync)       # SP#5
            k2t = loadc("k2t", 2, nc.sync)       # SP#6
            kmt = loadc("kmt", 4, nc.sync)       # SP#7
            ih5 = loadc("ih5", 4, nc.sync)       # SP#8
            tnt = loadc("tn", 2, nc.gpsimd)      # Pool#2

            ones = consts.tile([128, 1], BF16, tag="ones")
            nc.gpsimd.memset(ones, 1.0)
            # warm the ACT Square+Sqrt tables off the critical path
            # (~1.3us per set load). The warm input must be f32 — table
            # sets are input-dtype-specific and the real uses are f32.
            onesf = consts.tile([1, 1], F32, tag="onesf")
            nc.gpsimd.memset(onesf, 1.0)
            sqwarm = consts.tile([1, 2], F32, tag="sqwarm")
            nc.scalar.activation(
                out=sqwarm[:, 1:2], in_=onesf, func=ACTF.Sqrt, scale=1.0)

            # ---- FFT of [t | r]: re/im = FC/FS @ tr  (free dim 128) ----
            re_ps = psum.tile([128, 4, 128], F32, tag="re")
            im_ps = psum.tile([128, 4, 128], F32, tag="im")
            for ps, w in ((re_ps, fc8), (im_ps, fs8)):
                for ot in range(4):
                    for kc in range(2):
                        nc.tensor.matmul(
                            ps[:, ot, :], w[:, kc, ot * 128:(ot + 1) * 128],
                            tr[:, kc, :], start=(kc == 0), stop=(kc == 1))
            ure = re_ps[:, :, 0:BPC]
            rre = re_ps[:, :, BPC:2 * BPC]
            uim = im_ps[:, :, 0:BPC]
            rim = im_ps[:, :, BPC:2 * BPC]

            # ---- Engine legality on real HW: GPSIMD (Pool) cannot
            # access PSUM and supports only plain TensorTensor ops;
            # DVE/ACT may read one PSUM operand; DVE supports
            # scalar_tensor_tensor. Both FFT outputs are copied wholesale
            # to SBUF with the 1/8 prescale folded in, so all products
            # are SBUF-only plain muls (no PSUM-reader serialization).
            # Products/squares carry exact power-of-2 scales: reb/imb
            # hold [U/8 | R/8], so X*Y products are /64 as the B64/W64
            # constants expect. ----
            reb = state.tile([128, 4, 128], F32, tag="reb")
            nc.vector.tensor_scalar_mul(reb, re_ps, 0.125)
            # im side copied NEGATED (-1/8): t2's signs cancel, t4n's
            # minus comes for free, t3 restores its sign via the DVE stt
            # scalar — every Pool op stays a plain TensorTensor mul.
            sqim = state.tile([128, 4, BPC], BF16, tag="sqim")
            nc.scalar.activation(out=sqim, in_=uim, func=ACTF.Square,
                                 scale=0.125)
            imb = state.tile([128, 4, 128], F32, tag="imb")
            nc.scalar.activation(out=imb, in_=im_ps, func=ACTF.Copy,
                                 scale=-0.125)
            ureb = reb[:, :, 0:BPC]
            rreb = reb[:, :, BPC:2 * BPC]
            uimb = imb[:, :, 0:BPC]
            rimb = imb[:, :, BPC:2 * BPC]
            sqre = state.tile([128, 4, BPC], BF16, tag="sqre")
            nc.gpsimd.tensor_mul(sqre, ureb, ureb)
            t1 = state.tile([128, 4, BPC], BF16, tag="t1")
            t2 = state.tile([128, 4, BPC], BF16, tag="t2")
            t3 = state.tile([128, 4, BPC], BF16, tag="t3")
            t4n = state.tile([128, 4, BPC], BF16, tag="t4n")
            nc.gpsimd.tensor_mul(t1, ureb, rreb)
            nc.vector.scalar_tensor_tensor(
                out=t3, in0=uimb, scalar=-1.0, in1=rreb, op0=AL.mult,
                op1=AL.mult)
            lam64 = state.tile([128, 4, BPC], BF16, tag="lam64")
            nc.vector.scalar_tensor_tensor(
                out=lam64, in0=sqre, scalar=EPS / 64.0, in1=sqim,
                op0=AL.add, op1=AL.add)
            nc.gpsimd.tensor_mul(t2, uimb, rimb)
            nc.gpsimd.tensor_mul(t4n, ureb, rimb)

            # ---- lc = W64@lam64 first (lam64 ready early), then
            # bh = B64c@(t1+t2) + B64s@(t3+t4n) as one chain per gtile
            # (one open PSUM accumulation group per bank at a time);
            # late products (t2/t4n from Pool) ordered last per chain ----
            bha_ps = psum.tile([128, BPC], F32, tag="pb")
            bhb_ps = psum.tile([128, BPC], F32, tag="pg")
            lc_ps = psum.tile([128, 2, BPC], F32, tag="pa")
            for gt in range(2):
                for si, sq in enumerate((sqre, sqim)):
                    for kc in range(4):
                        nc.tensor.matmul(
                            lc_ps[:, gt, :],
                            w64[:, kc, gt * 128:(gt + 1) * 128],
                            sq[:, kc, :], start=(si == 0 and kc == 0),
                            stop=(si == 1 and kc == 3))
            terms = ((b64c, t1), (b64c, t2), (b64s, t3), (b64s, t4n))
            for ti, (w, t) in enumerate(terms):
                for gt, ps in ((0, bha_ps), (1, bhb_ps)):
                    for fc_ in range(4):
                        nc.tensor.matmul(
                            ps, w[:, fc_, gt * 128:(gt + 1) * 128],
                            t[:, fc_, :],
                            start=(ti == 0 and fc_ == 0),
                            stop=(ti == 3 and fc_ == 3))

            # ---- mu = 1/max(W@lam + cv, FLOOR) ----
            mu01 = state.tile([128, 2, BPC], F32, tag="mu01")
            mu = state.tile([128, 2, BPC], F32, tag="mu")
            for gt in range(2):
                nc.vector.tensor_scalar(
                    out=mu01[:, gt, :], in0=lc_ps[:, gt, :], scalar1=EPS,
                    scalar2=FLOOR, op0=AL.add, op1=AL.max)
                nc.vector.reciprocal(mu[:, gt, :], mu01[:, gt, :])

            # ---- x0 = K2(mu.bh); r0 = bh - 64 KM(lam64.x0); p0 = K2(mu.r0)

            def mm_k2(src_b, ptag):
                ps = psum.tile([128, 4, BPC], F32, tag=ptag)
                for ot in range(4):
                    for gc in range(2):
                        nc.tensor.matmul(
                            ps[:, ot, :],
                            k2t[:, gc, ot * 128:(ot + 1) * 128],
                            src_b[:, gc, :], start=(gc == 0), stop=(gc == 1))
                return ps

            def mm_km(src_b, ptag):
                ps = psum.tile([128, 2, BPC], F32, tag=ptag)
                for gt in range(2):
                    for fc_ in range(4):
                        nc.tensor.matmul(
                            ps[:, gt, :],
                            kmt[:, fc_, gt * 128:(gt + 1) * 128],
                            src_b[:, fc_, :], start=(fc_ == 0),
                            stop=(fc_ == 3))
                return ps

            sh0 = state.tile([128, 2, BPC], BF16, tag="sh0")
            nc.vector.tensor_mul(sh0[:, 0, :], mu[:, 0, :], bha_ps)
            nc.vector.tensor_mul(sh0[:, 1, :], mu[:, 1, :], bhb_ps)
            x0_ps = mm_k2(sh0, "pc")

            th0 = state.tile([128, 4, BPC], BF16, tag="th0")
            nc.vector.tensor_mul(th0, lam64, x0_ps)
            bhs = state.tile([128, 2, BPC], F32, tag="bhs")
            nc.scalar.copy(bhs[:, 0, :], bha_ps)
            nc.scalar.copy(bhs[:, 1, :], bhb_ps)
            x0s = state.tile([128, 4, BPC], F32, tag="x0s")
            nc.scalar.copy(x0s, x0_ps)
            g0_ps = mm_km(th0, "pa")
            rh = state.tile([128, 2, BPC], F32, tag="rh")
            nc.vector.scalar_tensor_tensor(
                out=rh, in0=g0_ps, scalar=-64.0, in1=bhs, op0=AL.mult,
                op1=AL.add)
            sh = state.tile([128, 2, BPC], BF16, tag="sh")
            nc.vector.tensor_mul(sh, mu, rh)
            p0_ps = mm_k2(sh, "pd")

            # ---- one full iteration (constant scalars) ----
            th = state.tile([128, 4, BPC], BF16, tag="th")
            nc.vector.tensor_mul(th, lam64, p0_ps)
            p0s = state.tile([128, 4, BPC], F32, tag="p0s")
            nc.scalar.copy(p0s, p0_ps)
            gh_ps = mm_km(th, "pa")
            rh2 = state.tile([128, 2, BPC], F32, tag="rh2")
            nc.vector.scalar_tensor_tensor(
                out=rh2, in0=gh_ps, scalar=-64.0 * AL0, in1=rh, op0=AL.mult,
                op1=AL.add)
            sh2 = state.tile([128, 2, BPC], BF16, tag="sh2")
            nc.vector.tensor_mul(sh2, mu, rh2)
            k2_ps = mm_k2(sh2, "pf")

            # xh2 = x0 + (a0 + a1 b1) p0 on Pool, off the critical path
            xh2 = state.tile([128, 4, BPC], F32, tag="xh2")
            nc.gpsimd.tensor_scalar_mul(xh2, p0s, XS)
            nc.gpsimd.tensor_add(xh2, xh2, x0s)

            u = state.tile([128, 4, BPC], BF16, tag="u")
            nc.vector.scalar_tensor_tensor(
                out=u, in0=k2_ps, scalar=AL1, in1=xh2, op0=AL.mult,
                op1=AL.add)

            # ---- finale: vN = IH5^T u (n-partition layout), ratios ----
            vN_ps = psum.tile([128, 2, BPC], F32, tag="im")
            for nt in range(2):
                for fc_ in range(4):
                    nc.tensor.matmul(
                        vN_ps[:, nt, :],
                        ih5[:, fc_, nt * 128:(nt + 1) * 128], u[:, fc_, :],
                        start=(fc_ == 0), stop=(fc_ == 3))
            sqD = state.tile([128, 2, BPC], BF16, tag="sqD")
            nc.scalar.activation(out=sqD, in_=vN_ps, func=ACTF.Square,
                                 scale=1.0)
            # sqT = (T.v)^2 = T^2 * sqD — stays on DVE, reads SBUF
            sqT = state.tile([128, 2, BPC], BF16, tag="sqT")
            for nt in range(2):
                nc.vector.tensor_scalar(
                    out=sqT[:, nt, :], in0=sqD[:, nt, :],
                    scalar1=tnt[:, nt, :], scalar2=None, op0=AL.mult)
            den2_ps = psum.tile([1, BPC], F32, tag="pc")
            num2_ps = psum.tile([1, BPC], F32, tag="pa")
            for nt in range(2):
                nc.tensor.matmul(den2_ps, ones, sqD[:, nt, :],
                                 start=(nt == 0), stop=(nt == 1))
            for nt in range(2):
                nc.tensor.matmul(num2_ps, ones, sqT[:, nt, :],
                                 start=(nt == 0), stop=(nt == 1))
            iden = state.tile([1, BPC], F32, tag="iden")
            nc.vector.reciprocal(iden, den2_ps)
            rat = state.tile([1, BPC], F32, tag="rat")
            nc.vector.tensor_mul(rat, num2_ps, iden)
            srat = state.tile([1, BPC], F32, tag="srat")
            nc.scalar.activation(out=srat, in_=rat, func=ACTF.Sqrt,
                                 scale=0.25)
            nc.sync.dma_start(out=d_out.ap(), in_=srat)

    nc.finalize()
    return nc


def _pack_inputs(recon, target):
    """Per-core [128, 256] bf16 DMA payloads: inputs prescaled by 1/8
    (exact) to match the x8 FFT weight prescale; partition p row c holds
    [target[:, c*128+p] | recon[:, c*128+p]]."""
    bf16 = _bf16np()
    outs = []
    for c in range(NCORES):
        sl = slice(c * BPC, (c + 1) * BPC)
        tt = (target[sl].astype(np.float32) * 0.125).astype(bf16)
        rr = (recon[sl].astype(np.float32) * 0.125).astype(bf16)
        tr3 = np.empty((128, 2, 2 * BPC), dtype=bf16)
        for kc in range(2):
            tr3[:, kc, 0:BPC] = tt[:, kc * 128:(kc + 1) * 128].T
            tr3[:, kc, BPC:2 * BPC] = rr[:, kc * 128:(kc + 1) * 128].T
        outs.append(np.ascontiguousarray(tr3.reshape(128, 2 * 128)))
    return outs


def kernel(recon: np.ndarray, target: np.ndarray) -> np.ndarray:
    from concourse.bass_utils import run_bass_kernel_spmd

    consts = _host_consts()
    nc = _program()

    trhs = _pack_inputs(recon, target)
    in_maps = []
    for c in range(NCORES):
        m = dict(consts)
        m["trh"] = trhs[c]
        in_maps.append(m)

    res = run_bass_kernel_spmd(nc, in_maps, core_ids=list(range(NCORES)))
    kernel._last_results = res  # for test.py introspection (profiling)
    total = 0.0
    for c in range(NCORES):
        total += float(res.results[c]["out"].astype(np.float64).sum())
    return np.float32(total)
